# revision 89
# baseline (speedup 1.0000x reference)
"""Trainium2 Bass kernel for 16-head causal MultiHeadAttention.

Problem shapes (hardcoded): x [4, 2048, 1024], Wq/Wk/Wv/Wo [1024, 1024],
bo [1024]. 16 heads, head_dim 64, causal, softmax scale 1/8.

Sharding: tensor-parallel over heads. Core c owns heads {2c, 2c+1}, i.e.
feature slice [128c : 128c+128] of the QKV projections and the matching
input rows of the output projection. The all-reduce over cores (and the
bias add) is done host-side on the 8 partial outputs.

Schedule (per core): batches are software-pipelined. The attention loop
of batch b interleaves, as PE "filler" work, the q/k projections and
natural-layout V computation of batch b+1 plus the output-projection
chunks of batch b (emitted right after each query chunk's normalize).
This keeps the PE busy while the softmax exp runs on the scalar engine.

Key device-level choices:
  - scores for the two heads are emitted as a row-tiled pair (contraction
    64 at array rows 0:64 / 64:128) -> they execute concurrently in the
    PE array (~2x).
  - exp is one ACT instruction per key-block covering both heads
    ([128, 2, 512] PSUM slab), amortizing the per-call overhead.
  - causal diagonal blocks are trimmed: scores/exp/ctx only touch
    q-columns >= 128*j; a single [128,128] triangle mask handles the
    remaining partial block.
  - V is computed directly in natural [kpos, feat] layout by using the
    x^T chunks as the stationary operand (no PE transposes), with a ones
    column appended per head so the ctx matmul also accumulates softmax
    denominators.
"""

import numpy as np

B, S, D, H = 4, 2048, 1024, 16
HD = D // H  # 64
N_CORES = 8
ROWS = B * S  # 8192
QC = 512  # query chunk
KC = 128  # key block
NQ = S // QC  # 4 query chunks per batch
NKB = S // KC  # 16 key blocks per batch

_cache = {}


def _build():
    import concourse.bacc as bacc
    import concourse.tile as tile
    from concourse import mybir

    fp32 = mybir.dt.float32
    bf16 = mybir.dt.bfloat16
    EXP = mybir.ActivationFunctionType.Exp

    nc = bacc.Bacc("TRN2", target_bir_lowering=False)

    xt_d = nc.dram_tensor("xt", [128, 8, ROWS], bf16, kind="ExternalInput")
    wq_d = nc.dram_tensor("wqt", [128, 8, 128], bf16, kind="ExternalInput")
    wk_d = nc.dram_tensor("wkt", [128, 8, 128], bf16, kind="ExternalInput")
    wv_d = nc.dram_tensor("wvt", [128, 8, 128], bf16, kind="ExternalInput")
    wo_d = nc.dram_tensor("wot", [128, 8, 128], bf16, kind="ExternalInput")
    mask_d = nc.dram_tensor("masks", [128, 2, 128], bf16, kind="ExternalInput")
    out_d = nc.dram_tensor("outp", [128, 8, ROWS], bf16, kind="ExternalOutput")

    with tile.TileContext(nc) as tc:
        with (
            tc.tile_pool(name="const", bufs=1) as const_pool,
            tc.tile_pool(name="xt", bufs=5) as xt_pool,
            tc.tile_pool(name="big", bufs=2) as big_pool,
            tc.tile_pool(name="at", bufs=8) as at_pool,
            tc.tile_pool(name="ot", bufs=6) as ot_pool,
            tc.tile_pool(name="small", bufs=6) as small_pool,
            tc.tile_pool(name="ps", bufs=2, space="PSUM") as sc_pool,
            tc.tile_pool(name="pc", bufs=1, space="PSUM") as ctx_pool,
            tc.tile_pool(name="pf", bufs=2, space="PSUM") as pf_pool,
        ):
            wq_sb = const_pool.tile([128, 8, 128], bf16, tag="wq")
            wk_sb = const_pool.tile([128, 8, 128], bf16, tag="wk")
            wv_sb = const_pool.tile([128, 8, 128], bf16, tag="wv")
            wo_sb = const_pool.tile([128, 8, 128], bf16, tag="wo")
            mask_sb = const_pool.tile([128, 2, 128], bf16, tag="mask")
            def dma_consts():
                # split transfers so the first proj unit's dependencies (wk
                # o-chunks 0-3) are small and land first on the fair-shared
                # DMA rings; the rest follows
                nc.sync.dma_start(wk_sb[:, 4:8, :], wk_d[:, 4:8, :])
                nc.sync.dma_start(wq_sb[:, 0:4, :], wq_d[:, 0:4, :])
                nc.sync.dma_start(wq_sb[:, 4:8, :], wq_d[:, 4:8, :])
                nc.sync.dma_start(wv_sb[:], wv_d[:])
                nc.sync.dma_start(wo_sb[:], wo_d[:])
                nc.sync.dma_start(mask_sb[:], mask_d[:])

            state = {}

            def dma_xt(b, rc, pieces=2):
                t = xt_pool.tile([128, 8, QC], bf16, tag="xt", name=f"xt_{b}_{rc}")
                g0 = b * S + rc * QC
                # split transfers: consumers of the first o-chunks can start
                # before the whole tile has landed
                w = 8 // pieces
                for p in range(pieces):
                    nc.sync.dma_start(
                        t[:, p * w : (p + 1) * w, :],
                        xt_d[:, p * w : (p + 1) * w, g0 : g0 + QC],
                    )
                state[("xt", b, rc)] = t

            def alloc_batch(b):
                state[("q", b)] = big_pool.tile([128, S], bf16, tag="qT", name=f"qT{b}")
                state[("k", b)] = big_pool.tile([128, S], bf16, tag="kT", name=f"kT{b}")
                state[("v", b)] = big_pool.tile(
                    [128, NKB, 2, 65], bf16, tag="vn", name=f"vn{b}"
                )
                state[("c", b)] = big_pool.tile(
                    [128, S], bf16, tag="ctxT", name=f"ctxT{b}"
                )
                # ones column for the softmax denominator
                nc.vector.memset(state[("v", b)][:, :, :, 64], 1.0)

            def proj_unit(b, rc, which):
                """qT/kT[:, rc-chunk] = W_slice @ x^T for batch b."""
                xt_sb = state[("xt", b, rc)]
                w_sb = wq_sb if which == "q" else wk_sb
                pf = pf_pool.tile([128, QC], fp32, tag="pf")
                for o in range(8):
                    nc.tensor.matmul(
                        pf[:],
                        w_sb[:, o, :],
                        xt_sb[:, o, :],
                        start=(o == 0),
                        stop=(o == 7),
                    )
                dst = state[(which, b)]
                nc.vector.tensor_copy(dst[:, rc * QC : (rc + 1) * QC], pf[:])

            def vnat_unit(b, rc):
                """v_nat rows [rc*512, rc*512+512) for batch b, both heads."""
                xt_sb = state[("xt", b, rc)]
                pf = pf_pool.tile([128, QC], fp32, tag="pf")
                for r in range(4):
                    for o in range(8):
                        nc.tensor.matmul(
                            pf[:, r * 128 : (r + 1) * 128],
                            xt_sb[:, o, r * 128 : (r + 1) * 128],
                            wv_sb[:, o, :],
                            start=(o == 0),
                            stop=(o == 7),
                        )
                vn = state[("v", b)]
                pv = pf[:].rearrange("p (r h f) -> p r h f", r=4, h=2)
                k0 = rc * 4
                nc.vector.tensor_copy(vn[:, k0 : k0 + 4, :, 0:64], pv[:])

            IDENT = mybir.ActivationFunctionType.Identity

            def outproj_unit(b, qi, mos=range(8), cast_split=False):
                """Partial out-projection for query chunk qi of batch b."""
                ctxT = state[("c", b)]
                g0 = b * S + qi * QC
                for mo in mos:
                    pf = pf_pool.tile([128, QC], fp32, tag="pf")
                    nc.tensor.matmul(
                        pf[:],
                        wo_sb[:, mo, :],
                        ctxT[:, qi * QC : (qi + 1) * QC],
                        start=True,
                        stop=True,
                    )
                    ot = ot_pool.tile([128, QC], bf16, tag="ot")
                    # offload some PSUM->SBUF casts to the scalar engine
                    on_act = mo % 2 == 1 if cast_split else mo % 3 == 2
                    if on_act:
                        nc.scalar.activation(ot[:], pf[:], IDENT, scale=1.0)
                    else:
                        nc.vector.tensor_copy(ot[:], pf[:])
                    nc.sync.dma_start(out_d[:, mo, g0 : g0 + QC], ot[:])

            def dummy_unit(b, n=4):
                """Throwaway matmuls that keep the PE active (HAM warm)
                across a stall with no real work available."""
                qT = state[("q", b)]
                for i in range(n):
                    pf = pf_pool.tile([128, QC], fp32, tag="pf")
                    nc.tensor.matmul(
                        pf[:], wq_sb[:, i % 8, :], qT[:, 0:QC], start=True, stop=True
                    )

            def make_fillers(b, rcs=range(4)):
                """Filler units preparing batch b (proj + v_nat)."""
                units = []
                for rc in rcs:
                    units.append(lambda b=b, rc=rc: proj_unit(b, rc, "k"))
                    units.append(lambda b=b, rc=rc: proj_unit(b, rc, "q"))
                    units.append(lambda b=b, rc=rc: vnat_unit(b, rc))
                return units

            def attention(b, fillers, pending_outproj, lead=()):
                """Causal attention for batch b; interleaves fillers and the
                pending outproj chunks. `lead` units are emitted first: ready
                PE work that bridges the previous batch's normalize tail."""
                qT, kT, vn = state[("q", b)], state[("k", b)], state[("v", b)]
                ctxT = state[("c", b)]
                for u in lead:
                    u()
                # fillers bridge normalize latency at chunk boundaries and
                # otherwise spread across the kc iterations with the outproj
                # half-units (halved so DVE cast bursts stay short)
                fi = 0
                n_iters = sum(4 * qi + 4 for qi in range(NQ))
                n_units = len(fillers) + 8
                it = 0
                emitted = 0
                # (b, qi, mos, it_ready): delay outproj a few iterations past
                # its normalize so the PE doesn't head-of-line block the DVE
                po = [(bb, qq, mos, 0) for (bb, qq, mos) in pending_outproj]
                def emit_ctx(pcs, vn, item):
                    kc, q0, at, is_last = item
                    for h in range(2):
                        nc.tensor.matmul(
                            pcs[h][0:65, q0:],
                            vn[:, kc, h, :],
                            at[:, h, q0:],
                            start=(kc == 0),
                            stop=is_last,
                        )

                for qi in range(NQ):
                    pcs = [
                        ctx_pool.tile(
                            [128, QC], fp32, tag=f"pc{h}", name=f"pc{h}_{b}_{qi}"
                        )
                        for h in range(2)
                    ]
                    kc_hi = 4 * qi + 4
                    # 2-deep software pipeline: ctx lags scores/exp by two
                    # iterations so a late exp slab never stalls the PE
                    ctx_q = []
                    for kc in range(kc_hi):
                        j = kc - 4 * qi
                        q0 = 128 * j if j > 0 else 0
                        ps = sc_pool.tile([128, 2, QC], fp32, tag="ps")
                        for h in range(2):
                            hs = slice(h * HD, (h + 1) * HD)
                            nc.tensor.matmul(
                                ps[:, h, q0:],
                                kT[hs, kc * KC : (kc + 1) * KC],
                                qT[hs, qi * QC + q0 : (qi + 1) * QC],
                                start=True,
                                stop=True,
                            )
                        at = at_pool.tile([128, 2, QC], bf16, tag="at")
                        nc.scalar.activation(at[:, :, q0:], ps[:, :, q0:], EXP, scale=0.125)
                        if j >= 0:
                            # one fused mul masks the triangle for both heads
                            nc.vector.tensor_mul(
                                at[:, :, q0 : q0 + 128],
                                at[:, :, q0 : q0 + 128],
                                mask_sb[:],
                            )
                        ctx_q.append((kc, q0, at, kc == kc_hi - 1))
                        # filler slot BEFORE the lagged ctx: the filler is
                        # dependency-free, so if ctx's exp is late the PE
                        # chews the filler instead of head-of-line blocking
                        it += 1
                        if it * n_units // n_iters > emitted:
                            if po and it >= po[0][3]:
                                bb, qq, mos, _ = po.pop(0)
                                outproj_unit(bb, qq, mos)
                                emitted += 1
                            elif fi < len(fillers):
                                fillers[fi]()
                                fi += 1
                                emitted += 1
                            else:
                                # no real work ready: short warm-keeper so
                                # the PE doesn't underrun in this stretch
                                dummy_unit(b, 2)
                                emitted += 1
                        if len(ctx_q) > 2:
                            emit_ctx(pcs, vn, ctx_q.pop(0))
                    # flush the remaining ctx for this query chunk
                    while ctx_q:
                        emit_ctx(pcs, vn, ctx_q.pop(0))
                    # normalize query chunk qi. Both rowsum copies go first,
                    # back-to-back on the scalar engine (idle at chunk
                    # boundaries), so head 1's DVE/gpsimd chain isn't
                    # serialized behind head 0's — each head's PSUM
                    # accumulator frees as early as possible.
                    rss = []
                    for h in range(2):
                        rs = small_pool.tile([1, QC], fp32, tag=f"rs{h}")
                        nc.scalar.activation(rs[0:1, :], pcs[h][64:65, :], IDENT, scale=1.0)
                        rss.append(rs)
                    for h in range(2):
                        rr = small_pool.tile([1, QC], fp32, tag=f"rr{h}")
                        nc.vector.reciprocal_approx_fast(rr[:], rss[h][:])
                        rb = small_pool.tile([HD, QC], fp32, tag=f"rb{h}")
                        nc.gpsimd.partition_broadcast(rb[:], rr[0:1, :])
                        nc.vector.tensor_mul(
                            ctxT[h * HD : (h + 1) * HD, qi * QC : (qi + 1) * QC],
                            pcs[h][0:64, :],
                            rb[:],
                        )
                    po.append((b, qi, range(0, 4), it + 2))
                    po.append((b, qi, range(4, 8), it + 4))
                    # bridge the normalize latency with pure-PE filler work so
                    # the PE never idles long enough for HAM to re-throttle.
                    # Emitted AFTER the normalize chain: the fillers' casts
                    # queue behind it on the DVE, the filler matmuls run on
                    # the PE while the DVE works the normalize.
                    for k in range(2):
                        if fi < len(fillers):
                            fillers[fi]()
                            fi += 1
                            emitted += 1
                        elif po and it >= po[0][3]:
                            bb, qq, mos, _ = po.pop(0)
                            outproj_unit(bb, qq, mos)
                        elif k == 0:
                            dummy_unit(b)
                # drain remaining fillers, keep the last outproj pending
                while fi < len(fillers):
                    fillers[fi]()
                    fi += 1
                while len(po) > 2:
                    bb, qq, mos, _ = po.pop(0)
                    outproj_unit(bb, qq, mos)
                return [(bb, qq, mos) for (bb, qq, mos, _) in po]

            # ---- prologue: batch 0 projections ----
            # Stage the DMAs: each dma_start shards across all 16 rings, so
            # queueing everything at once makes the first-needed chunk finish
            # last. Issue chunk rc only right before its compute units.
            # the first proj unit needs wk chunk 0 AND xt(0,0) chunk 0 —
            # queue the small weight piece first on the fair-shared rings
            nc.sync.dma_start(wk_sb[:, 0:4, :], wk_d[:, 0:4, :])
            dma_xt(0, 0, pieces=8)
            dma_consts()
            alloc_batch(0)
            for rc in range(4):
                if rc > 0:
                    dma_xt(0, rc)
                for u in make_fillers(0, rcs=[rc]):
                    u()

            pending: list = []
            lead: list = []
            for b in range(B):
                if b < B - 1:
                    for rc in range(4):
                        dma_xt(b + 1, rc)
                    alloc_batch(b + 1)
                    if b == B - 2:
                        # split the last batch's prep: half now, half as
                        # fillers inside its own attention (which otherwise
                        # has no filler work and starves the PE)
                        fillers = make_fillers(b + 1, rcs=[0, 1])
                    else:
                        fillers = make_fillers(b + 1)
                    # hold the last two units back: they run at the TOP of
                    # the next attention as ready PE work bridging the
                    # batch handoff (vnat(rc3) and q-proj(rc3) — both only
                    # consumed late in that batch's own attention)
                    next_lead = [fillers.pop(), fillers.pop(-2)]
                elif b == B - 1:
                    fillers = make_fillers(b, rcs=[2, 3])
                    next_lead = []
                pending = attention(b, fillers, pending, lead)
                lead = next_lead
            # bridge the final normalize with warm-keeper matmuls so the
            # closing outproj runs at full clock
            dummy_unit(B - 1, n=14)
            for bb, qq, mos in pending:
                outproj_unit(bb, qq, mos, cast_split=True)

    nc.compile()
    return nc


def _prep_inputs(x, Wq, Wk, Wv, Wo, bo):
    import ml_dtypes

    bf = ml_dtypes.bfloat16

    x = np.ascontiguousarray(np.asarray(x, dtype=np.float32))
    Wq = np.asarray(Wq, dtype=np.float32)
    Wk = np.asarray(Wk, dtype=np.float32)
    Wv = np.asarray(Wv, dtype=np.float32)
    Wo = np.asarray(Wo, dtype=np.float32)

    x_flat = x.reshape(ROWS, D)
    # xt[p, o, n] = x_flat[n, 128*o + p]
    xt = np.ascontiguousarray(
        x_flat.T.reshape(8, 128, ROWS).transpose(1, 0, 2).astype(bf)
    )

    # triangle mask for the diagonal [128,128] sub-block: k <= q
    # (duplicated along a head axis so one mul covers both heads)
    karr = np.arange(128)[:, None]
    qarr = np.arange(128)[None, :]
    tri = (karr <= qarr).astype(bf)
    mask = np.ascontiguousarray(np.stack([tri, tri], axis=1))

    in_maps = []
    for c in range(N_CORES):
        sl = slice(128 * c, 128 * c + 128)

        def wt(W):
            # lhsT chunks: [p(=d within chunk), o(=D chunk), m(=slice feat)]
            Ws = W[sl, :]  # [128, 1024]
            return np.ascontiguousarray(
                Ws.T.reshape(8, 128, 128).transpose(1, 0, 2).astype(bf)
            )

        # wot[f, mo, d] = Wo[128*mo + d, 128*c + f]
        wot = np.ascontiguousarray(
            Wo[:, sl].reshape(8, 128, 128).transpose(2, 0, 1).astype(bf)
        )
        in_maps.append(
            {
                "xt": xt,
                "wqt": wt(Wq),
                "wkt": wt(Wk),
                "wvt": wt(Wv),
                "wot": wot,
                "masks": mask,
            }
        )
    return in_maps


def _run(in_maps, trace=False):
    from concourse.bass_utils import run_bass_kernel_spmd

    if "nc" not in _cache:
        _cache["nc"] = _build()
    return run_bass_kernel_spmd(
        _cache["nc"], in_maps, core_ids=list(range(N_CORES)), trace=trace
    )


def kernel(x, Wq, Wk, Wv, Wo, bo, _trace=False):
    in_maps = _prep_inputs(x, Wq, Wk, Wv, Wo, bo)
    res = _run(in_maps, trace=_trace)
    acc = np.zeros((128, 8, ROWS), dtype=np.float32)
    for r in res.results:
        acc += r["outp"]
    out = acc.transpose(2, 1, 0).reshape(ROWS, D)
    out = out + np.asarray(bo, dtype=np.float32)[None, :]
    out = out.reshape(B, S, D)
    if _trace:
        kernel.last_exec_time_ns = res.exec_time_ns
    return out
